# revision 18
# baseline (speedup 1.0000x reference)
"""Trainium2 Bass kernel for ComplexAttentionLayer.

Problem: complex-valued self-attention, B=8, L=1024, D=512, H=8 heads,
Dh=64, q=k=v = input. Returns (out_re, out_im, attn_re, attn_im) where
attn is the head-mean of the complex softmax weights.

Sharding: data-parallel over batch across the 8 NeuronCores (one batch
element per core); all 8 heads computed locally per core, no
collectives.

Math exploited per (b, h)  (q = k = v):
  s_re = qre@qreT - qim@qimT     (symmetric)
  s_im = qre@qimT + qim@qreT     (symmetric)
  E_*  = exp(0.125 * s_*)        (also symmetric),  Z_* = rowsum(E_*)
  a_*  = E_* / Z_*  (row scaling)
  o_re = diag(1/Zre) (E_re@qre) - diag(1/Zim) (E_im@qim)
  o_im = diag(1/Zre) (E_re@qim) + diag(1/Zim) (E_im@qre)
  attn_* = (1/8) sum_h E_* / Z_*

Because E is symmetric, the stored [l, s] tiles of E serve directly as
the K-major (s-on-partitions) operand of the second matmul -- no
transpose of the attention weights is ever needed.
"""

import sys
import numpy as np

if "/opt/trn_rl_repo" not in sys.path:
    sys.path.insert(0, "/opt/trn_rl_repo")

B, L, D, H = 8, 1024, 512, 8
DH = D // H          # 64
NC_ = 8              # l-chunks of 128
SCALE = 1.0 / 8.0    # 1/sqrt(Dh)
EXP_BIAS = -8.0      # keeps exp() in fp16 range; cancels in softmax

_built = {}


def _build_program():
    """Build the single-core Bass/Tile program (same program on all 8 cores)."""
    import concourse.bass as bass
    import concourse.tile as tile
    from concourse import bacc, mybir
    from contextlib import ExitStack

    f32 = mybir.dt.float32
    f16 = mybir.dt.float16
    ts = bass.ts
    AF = mybir.ActivationFunctionType
    ALU = mybir.AluOpType

    nc = bacc.Bacc("TRN2", target_bir_lowering=False, debug=False)

    x_re_d = nc.dram_tensor("x_re", [L, D], f32, kind="ExternalInput").ap()
    x_im_d = nc.dram_tensor("x_im", [L, D], f32, kind="ExternalInput").ap()
    id16_d = nc.dram_tensor("ident16", [128, 128], f16, kind="ExternalInput").ap()
    id32_d = nc.dram_tensor("ident32", [128, 128], f32, kind="ExternalInput").ap()
    out_re_d = nc.dram_tensor("out_re", [L, D], f32, kind="ExternalOutput").ap()
    out_im_d = nc.dram_tensor("out_im", [L, D], f32, kind="ExternalOutput").ap()
    attn_re_d = nc.dram_tensor("attn_re", [L, L], f32, kind="ExternalOutput").ap()
    attn_im_d = nc.dram_tensor("attn_im", [L, L], f32, kind="ExternalOutput").ap()

    with tile.TileContext(nc) as tc, ExitStack() as ctx:
        const = ctx.enter_context(tc.tile_pool(name="const", bufs=1))
        persist = ctx.enter_context(tc.tile_pool(name="persist", bufs=1))
        epool = ctx.enter_context(tc.tile_pool(name="E", bufs=32))
        zpool = ctx.enter_context(tc.tile_pool(name="z", bufs=2))
        tpsb_pool = ctx.enter_context(tc.tile_pool(name="tpsb", bufs=1))
        small = ctx.enter_context(tc.tile_pool(name="small", bufs=4))
        attnf_pool = ctx.enter_context(tc.tile_pool(name="attnf", bufs=1))
        wpool = ctx.enter_context(tc.tile_pool(name="wpool", bufs=2))

        ebias = const.tile([128, 1], f32, tag="ebias", name="ebias")
        nc.vector.memset(ebias[:], EXP_BIAS)
        id16 = const.tile([128, 128], f16, tag="id16", name="id16")
        nc.sync.dma_start(id16[:], id16_d)
        id32 = const.tile([128, 128], f32, tag="id32", name="id32")
        nc.sync.dma_start(id32[:], id32_d)

        # ---------------- prologue: load, cast to fp16, transpose ----------
        # xb_* : natural [l, d] layout, fp16 (weights of the 2nd matmul),
        #        one big tile each, l-chunk c at cols [c*512, (c+1)*512)
        # xt_* : transposed [d, l] layout, fp16 (operands of the 1st matmul)
        xb_re = persist.tile([128, NC_ * D], f16, tag="xbre", name="xbre")
        xb_im = persist.tile([128, NC_ * D], f16, tag="xbim", name="xbim")
        xb_nim = persist.tile([128, NC_ * D], f16, tag="xbnim", name="xbnim")
        with tc.tile_pool(name="xld", bufs=1) as xld, \
             tc.tile_pool(name="ptr", bufs=2, space="PSUM") as ps_tr:
            # one DMA per input: DRAM [1024, 512] -> SBUF [128, 8*512]
            # (partition p, col c*512+j) <- x[c*128+p, j]
            t_re = xld.tile([128, NC_ * D], f32, tag="ldre", name="ldre")
            nc.sync.dma_start(
                t_re[:].rearrange("p (c j) -> p c j", j=D),
                x_re_d.rearrange("(c p) j -> p c j", p=128),
            )
            t_im = xld.tile([128, NC_ * D], f32, tag="ldim", name="ldim")
            nc.sync.dma_start(
                t_im[:].rearrange("p (c j) -> p c j", j=D),
                x_im_d.rearrange("(c p) j -> p c j", p=128),
            )
            for c in range(NC_):
                nc.scalar.mul(xb_re[:, ts(c, D)], t_re[:, ts(c, D)], 1.0)
                nc.scalar.mul(xb_im[:, ts(c, D)], t_im[:, ts(c, D)], 1.0)
                nc.scalar.mul(xb_nim[:, ts(c, D)], t_im[:, ts(c, D)], -1.0)

            xt_re, xt_im, xt_nim = [], [], []
            for name, src, dst in (
                ("xtre", xb_re, xt_re),
                ("xtim", xb_im, xt_im),
                ("xtnim", xb_nim, xt_nim),
            ):
                for g in range(4):  # d-groups of 128
                    ptile = ps_tr.tile([128, 1024], f16, tag="ptr", name="ptr")
                    for c in range(NC_):
                        nc.tensor.transpose(
                            ptile[:, ts(c, 128)],
                            src[:, c * D + g * 128 : c * D + (g + 1) * 128],
                            id16[:],
                        )
                    xt = persist.tile([128, 1024], f16, tag=f"{name}{g}", name=f"{name}{g}")
                    nc.vector.tensor_copy(xt[:], ptile[:])
                    dst.append(xt)

        # main-loop PSUM pools (opened after the prologue pool is freed)
        ps_score = ctx.enter_context(tc.tile_pool(name="pscore", bufs=2, space="PSUM"))
        ps_tp = ctx.enter_context(tc.tile_pool(name="ptp", bufs=2, space="PSUM"))
        ps_tt = ctx.enter_context(tc.tile_pool(name="ptt", bufs=2, space="PSUM"))

        # attn-mean accumulators, fp16
        acc = [
            [persist.tile([128, 1024], f16, tag=f"acc{comp}_{c}", name=f"acc{comp}_{c}") for c in range(NC_)]
            for comp in range(2)
        ]

        # ---------------- main loop over head pairs ------------------------
        Etiles = {}
        for hp in range(4):
            heads = (2 * hp, 2 * hp + 1)
            Z = {}
            for h in heads:
                for comp in range(2):
                    Z[(h, comp)] = zpool.tile([128, 8], f32, tag=f"Z{h % 2}_{comp}", name=f"Z{h % 2}_{comp}")

            # -------- scores + exp, chunk by chunk, heads interleaved ------
            for c in range(NC_):
                for comp in range(2):
                    pscore = {}
                    for h in heads:
                        pscore[h] = ps_score.tile([128, 1024], f32, tag="score", name="score")
                    for sh in range(2):
                        for step in range(2):
                            for h in heads:
                                g, po = h // 2, (h % 2) * 64
                                if comp == 0:
                                    # s_re = qre.qre - qim.qim
                                    lhs = (xt_re, xt_im)[step][g]
                                    rhs = (xt_re, xt_nim)[step][g]
                                else:
                                    # s_im = qre.qim + qim.qre
                                    lhs = (xt_re, xt_im)[step][g]
                                    rhs = (xt_im, xt_re)[step][g]
                                nc.tensor.matmul(
                                    pscore[h][:, ts(sh, 512)],
                                    lhs[po : po + 64, ts(c, 128)],
                                    rhs[po : po + 64, ts(sh, 512)],
                                    start=(step == 0),
                                    stop=(step == 1),
                                )
                    for h in heads:
                        E = epool.tile([128, 1024], f16, tag="E", name="Et")
                        Etiles[(h, comp, c)] = E
                        nc.scalar.activation(
                            E[:],
                            pscore[h][:],
                            AF.Exp,
                            scale=SCALE,
                            bias=ebias[:],
                            accum_out=Z[(h, comp)][:, c : c + 1],
                        )

            # -------- per-head epilogue: softmax scale, out matmuls --------
            for h in heads:
                g, po = h // 2, (h % 2) * 64
                zre = zpool.tile([128, 8], f32, tag=f"zre{h % 2}", name=f"zre{h % 2}")
                nc.vector.reciprocal(zre[:], Z[(h, 0)][:])
                zim = zpool.tile([128, 8], f32, tag=f"zim{h % 2}", name=f"zim{h % 2}")
                nc.vector.reciprocal(zim[:], Z[(h, 1)][:])
                z8re = zpool.tile([128, 8], f32, tag=f"z8re{h % 2}", name=f"z8re{h % 2}")
                nc.vector.tensor_scalar_mul(z8re[:], zre[:], 1.0 / H)
                z8im = zpool.tile([128, 8], f32, tag=f"z8im{h % 2}", name=f"z8im{h % 2}")
                nc.vector.tensor_scalar_mul(z8im[:], zim[:], 1.0 / H)

                # attn accumulation: acc += E * (1/(H*Z)) (per-partition scalar)
                for comp in range(2):
                    z8 = (z8re, z8im)[comp]
                    for c in range(NC_):
                        E = Etiles[(h, comp, c)]
                        if h == 0:
                            nc.vector.tensor_scalar_mul(
                                acc[comp][c][:], E[:], z8[:, c : c + 1]
                            )
                        else:
                            nc.vector.scalar_tensor_tensor(
                                acc[comp][c][:],
                                E[:],
                                z8[:, c : c + 1],
                                acc[comp][c][:],
                                op0=ALU.mult,
                                op1=ALU.add,
                            )

                # 2nd matmul, transposed form: T^T[dpair, l] accumulated over
                # s-chunks.  family 0: E_re moving, weights [qre | qim]
                #            family 1: E_im moving, weights [-qim | qre]
                # Pre-concatenate per-head weight pairs: wp[fam] is [128, 1024]
                # fp16 where k-chunk k occupies cols [k*128, k*128+128) as
                # [w0_k(64) | w1_k(64)].
                tpsb = []
                for fam in range(2):
                    w0src = (xb_re, xb_nim)[fam]
                    w1src = (xb_im, xb_re)[fam]
                    wp = wpool.tile([128, 1024], f16, tag=f"wp{fam}", name=f"wp{fam}")
                    wp3 = wp[:].rearrange("p (k d) -> p k d", d=128)
                    for wsrc, dlo in ((w0src, 0), (w1src, DH)):
                        nc.vector.tensor_copy(
                            wp3[:, :, dlo : dlo + DH],
                            wsrc[:].rearrange("p (k d) -> p k d", d=D)[
                                :, :, ts(h, DH)
                            ],
                        )
                    tp_sb = tpsb_pool.tile([128, 1024], f32, tag=f"tpsb{fam}", name=f"tpsb{fam}")
                    for lh in range(2):
                        ptp = ps_tp.tile([128, 512], f32, tag="tp", name="tp")
                        for k in range(NC_):
                            nc.tensor.matmul(
                                ptp[:],
                                wp[:, ts(k, 128)],
                                Etiles[(h, fam, k)][:, ts(lh, 512)],
                                start=(k == 0),
                                stop=(k == NC_ - 1),
                            )
                        nc.vector.tensor_copy(tp_sb[:, ts(lh, 512)], ptp[:])
                    tpsb.append(tp_sb)

                # transpose back [dpair, l] -> [l, dpair], apply 1/Z, store
                for c in range(NC_):
                    ptt = ps_tt.tile([128, 256], f32, tag="tt", name="tt")
                    nc.tensor.transpose(ptt[:, 0:128], tpsb[0][:, ts(c, 128)], id32[:])
                    nc.tensor.transpose(ptt[:, 128:256], tpsb[1][:, ts(c, 128)], id32[:])
                    tmp = small.tile([128, 128], f32, tag="tmp", name="tmp")
                    nc.vector.tensor_scalar_mul(tmp[:], ptt[:, 0:128], zre[:, c : c + 1])
                    opair = small.tile([128, 128], f32, tag="opair", name="opair")
                    nc.vector.scalar_tensor_tensor(
                        opair[:], ptt[:, 128:256], zim[:, c : c + 1], tmp[:],
                        op0=ALU.mult, op1=ALU.add,
                    )
                    nc.sync.dma_start(
                        out_re_d[ts(c, 128), ts(h, DH)], opair[:, 0:DH]
                    )
                    nc.sync.dma_start(
                        out_im_d[ts(c, 128), ts(h, DH)], opair[:, DH:128]
                    )

        # ---------------- attn outputs: fp16 acc -> fp32, DMA --------------
        for comp, attn_d in ((0, attn_re_d), (1, attn_im_d)):
            for c in range(NC_):
                af = attnf_pool.tile([128, 1024], f32, tag="af", name="af")
                nc.vector.tensor_copy(af[:], acc[comp][c][:])
                nc.sync.dma_start(attn_d[ts(c, 128), :], af[:])

    nc.finalize()
    return nc


def _get_nc():
    if "nc" not in _built:
        _built["nc"] = _build_program()
    return _built["nc"]


def make_in_maps(x_re, x_im):
    id16 = np.eye(128, dtype=np.float16)
    id32 = np.eye(128, dtype=np.float32)
    return [
        {
            "x_re": np.ascontiguousarray(x_re[b]),
            "x_im": np.ascontiguousarray(x_im[b]),
            "ident16": id16,
            "ident32": id32,
        }
        for b in range(B)
    ]


def run_on_hw(x_re, x_im, trace=False):
    from concourse.bass_utils import run_bass_kernel_spmd

    nc = _get_nc()
    rr = run_bass_kernel_spmd(
        nc, make_in_maps(x_re, x_im), list(range(B)), trace=trace
    )
    results = rr.results
    out_re = np.stack([results[b]["out_re"] for b in range(B)]).astype(np.float32)
    out_im = np.stack([results[b]["out_im"] for b in range(B)]).astype(np.float32)
    attn_re = np.stack([results[b]["attn_re"] for b in range(B)]).astype(np.float32)
    attn_im = np.stack([results[b]["attn_im"] for b in range(B)]).astype(np.float32)
    return (out_re, out_im, attn_re, attn_im), rr


def time_on_hw(x_re, x_im, tries=3):
    """Upper-bound per-call wall time of the 8-core execution (includes
    host dispatch through the axon tunnel; device exec is a fraction)."""
    import time as _time

    run_on_hw(x_re, x_im, trace=False)  # warm: compile + cache
    best = float("inf")
    for _ in range(tries):
        t0 = _time.perf_counter()
        run_on_hw(x_re, x_im, trace=False)
        best = min(best, _time.perf_counter() - t0)
    return best * 1e9


def kernel(x_re, x_im):
    x_re = np.asarray(x_re, dtype=np.float32)
    x_im = np.asarray(x_im, dtype=np.float32)
    outs, _ = run_on_hw(x_re, x_im, trace=False)
    return outs
